# revision 12
# baseline (speedup 1.0000x reference)
"""Trainium2 Bass kernel for an attention block (B=4, T=2048, H=512, 8 heads).

Sharding: 8 cores = 4 batches x 2 head-groups (4 heads each). Core c handles
batch c//2 and heads [4*(c%2), 4*(c%2)+4) over the FULL 2048-token context.
Each core emits a PARTIAL output projection (its 256 attn features x its
w_proj row-slice); the host sums the two partials per batch and adds b_proj.

v2 schedule, built around two measured facts:
  - ScalarE exp over [4 heads, 2048 q, 2048 k] is 128 ACTIVATEs of [128,1024]
    at ~1.09us each = ~140us. That stream is the floor; nothing else may ever
    block it.
  - The PE HAM clock gate halves the PE clock whenever the PE array has idle
    slack in its 3.4us activity window. At 1.2GHz the bf16 score+PV work
    (2048 cols/unit) exceeds the exp period (1147ns) and the exp stream
    starves. The baseline oscillated between these states and lost 60-100us.

  Fixes:
  - PV runs as fp8e4 DoubleRow over key-tile PAIRS (contraction 256/pass):
    512 PE cols/unit instead of 1024. exp output is written fp8 directly by
    the ACT (free cast); V is drained to fp8 by the DVE (free cast). Scores
    stay bf16 (K=64 contraction gains nothing from fp8).
  - Block order [g0qh0 g1qh0 g0qh1 g1qh1 g2qh1 g3qh1 g2qh0 g3qh0] over
    1024-wide query halves: heads 0,1 finish all tokens by unit 64, so the
    c=0 half of the output projection streams as PE filler from unit 66; the
    c=1 half for tokens 1024-2047 streams from unit 97 (with DMA-out); only
    tokens 0-1023's c=1 half remains for the tail.
  - All remaining QKV projections drip as deadline-scheduled fills, ~1 per
    unit, spread across the whole stream instead of front-loaded bursts.
    Optional junk matmuls pad the thin late blocks to keep the HAM warm.
  - PSUM: 2 score bufs (4 banks) + [65,1024] accumulator (2 banks, ones-row
    softmax denominator trick) + fill ping-pong (2 banks) = all 8.
"""

import sys

sys.path.insert(0, "/opt/trn_rl_repo")

from contextlib import ExitStack

import numpy as np

import concourse.bass as bass
import concourse.tile as tile
from concourse import bacc, mybir

F32 = mybir.dt.float32
BF16 = mybir.dt.bfloat16
FP8 = mybir.dt.float8e4
Act = mybir.ActivationFunctionType
DR = mybir.MatmulPerfMode.DoubleRow

B, T, H = 4, 2048, 512
HEADS = 8
D = H // HEADS  # 64
GC = 4  # heads per core
HC = H // 128  # 4 x-feature chunks
NT = T // 128  # 16 key tiles
NTT = T // 512  # 4 token 512-chunks
SCALE = float(D) ** -0.5
DP = 68  # padded v row stride (fp8 bytes): 64 d + ones + pad, %16 == 0

USE_DR_PV = True  # fp8e4 DoubleRow PV (pairs of key tiles)
JUNK = 0  # junk matmuls per unit in fill-starved units (off: measured net loss)

# block order: (g, qh) over 1024-token query halves
ORDER = [(0, 0), (1, 0), (0, 1), (1, 1), (2, 1), (3, 1), (2, 0), (3, 0)]


def build_nc():
    nc = bacc.Bacc("TRN2", target_bir_lowering=False, debug=False, num_devices=8)

    x_in = nc.dram_tensor("x_in", [H, T], BF16, kind="ExternalInput").ap()  # x.T
    w_in = nc.dram_tensor("w_in", [H, 3 * 256], BF16, kind="ExternalInput").ap()
    # bias layout: [bq*SCALE (256) | bk (256)] fp32, bv (256) fp32
    b_in = nc.dram_tensor("b_in", [3 * 256], F32, kind="ExternalInput").ap()
    wp_in = nc.dram_tensor("wp_in", [256, H], BF16, kind="ExternalInput").ap()
    out = nc.dram_tensor("out", [T, H], BF16, kind="ExternalOutput").ap()

    with tile.TileContext(nc) as tc, ExitStack() as ctx:
        per = ctx.enter_context(tc.tile_pool(name="persist", bufs=1))

        w_sb = per.tile([128, HC, 3 * 256], BF16)
        wp_sb = per.tile([128, 2, H], BF16)
        bqk_sb = per.tile([128, 4], F32)  # q bias chunks 0-1, k bias chunks 2-3
        bv_row = per.tile([1, 256], F32)
        bv_bc = per.tile([128, 256], F32)
        dummy = per.tile([1, 8], F32)
        warm = per.tile([128, 128], BF16)

        xT = per.tile([128, HC, T], BF16)
        kT = [per.tile([128, T], BF16, name=f"kT_{j}") for j in range(2)]
        qT = [per.tile([128, T], BF16, name=f"qT_{j}") for j in range(2)]
        v_dt = FP8 if USE_DR_PV else BF16
        v_sb = per.tile([128, NT, GC, DP], v_dt)
        attnT = per.tile([128, 2, T], BF16)
        part_sb = per.tile([128, NT, 512], F32)  # c=0 out-proj partials

        # ---- phase A ----
        with (
            tc.tile_pool(name="qkps", bufs=4, space="PSUM") as qkps,
            tc.tile_pool(name="warmps", bufs=1, space="PSUM") as warmps,
        ):
            # x is pre-transposed on the host. Few, big DMAs: w first on scalar
            # (the first matmuls need it), x quarters split across queues.
            def x_dma(eng, clo, tlo, thi):
                eng.dma_start(
                    out=xT[:, clo : clo + 2, tlo:thi],
                    in_=x_in[128 * clo : 128 * (clo + 2), tlo:thi].rearrange(
                        "(c p) t -> p c t", p=128
                    ),
                )

            nc.scalar.dma_start(
                out=w_sb[:, :, 0:512],
                in_=w_in[:, 0:512].rearrange("(c p) j -> p c j", p=128),
            )
            nc.scalar.dma_start(
                out=w_sb[:, :, 512:768],
                in_=w_in[:, 512:768].rearrange("(c p) j -> p c j", p=128),
            )
            nc.gpsimd.dma_start(
                out=bqk_sb, in_=b_in[0:512].rearrange("(c p) -> p c", p=128)
            )
            nc.gpsimd.dma_start(out=bv_row, in_=b_in[512:768].unsqueeze(0))
            x_dma(nc.sync, 0, 0, 512)
            x_dma(nc.gpsimd, 2, 0, 512)
            x_dma(nc.sync, 0, 512, 1024)
            x_dma(nc.gpsimd, 2, 512, 1024)
            x_dma(nc.sync, 2, 1024, 2048)
            x_dma(nc.scalar, 0, 1024, 2048)
            nc.scalar.dma_start(
                out=wp_sb, in_=wp_in.rearrange("(c p) j -> p c j", p=128)
            )
            # junk matmuls warm the PE clock gate (HAM) during the DMA wait so
            # the first real projections run at 2.4 GHz
            nc.vector.memset(warm, 0.0)
            wps = warmps.tile([128, 128], F32, tag="w", name="warmps")
            for _ in range(48):
                nc.tensor.matmul(wps, lhsT=warm, rhs=warm, start=True, stop=True)
            # preload the exp table while DMAs stream
            nc.gpsimd.memset(dummy, 0.0)
            nc.scalar.activation(dummy, dummy, Act.Exp)
            nc.gpsimd.partition_broadcast(bv_bc, bv_row)
            nc.vector.memset(v_sb[:, :, :, D : D + 1], 1.0)

            # K/Q projection group: 4 matmuls + a drain; emitted in halves
            # (c-chunk pairs) so fills spread smoothly.
            def kq_mm(kind, jt, tt, ps, cs):
                col0 = 256 + 128 * jt if kind == "k" else 128 * jt
                for c in cs:
                    nc.tensor.matmul(
                        ps[:, 0:512],
                        lhsT=w_sb[:, c, col0 : col0 + 128],
                        rhs=xT[:, c, 512 * tt : 512 * (tt + 1)],
                        start=(c == 0),
                        stop=(c == HC - 1),
                    )

            def kq_drain(kind, jt, tt, ps):
                if kind == "k":
                    nc.vector.tensor_scalar(
                        out=kT[jt][:, 512 * tt : 512 * (tt + 1)],
                        in0=ps[:, 0:512],
                        scalar1=bqk_sb[:, 2 + jt : 3 + jt],
                        scalar2=None,
                        op0=mybir.AluOpType.add,
                    )
                else:
                    nc.vector.tensor_scalar(
                        out=qT[jt][:, 512 * tt : 512 * (tt + 1)],
                        in0=ps[:, 0:512],
                        scalar1=bqk_sb[:, jt : jt + 1],
                        scalar2=SCALE,
                        op0=mybir.AluOpType.add,
                        op1=mybir.AluOpType.mult,
                    )

            # V for head range [2*ghalf, 2*ghalf+2) over `ks` 128-token tiles
            # of 512-chunk tt: 4 matmuls per tile, one strided drain.
            def v_group(tt, ghalf, ps, ks):
                for k in ks:
                    i = 4 * tt + k
                    for c in range(HC):
                        nc.tensor.matmul(
                            ps[:, 128 * (k % 4) : 128 * (k % 4) + 128],
                            lhsT=xT[:, c, 128 * i : 128 * (i + 1)],
                            rhs=w_sb[:, c, 512 + 128 * ghalf : 512 + 128 * (ghalf + 1)],
                            start=(c == 0),
                            stop=(c == HC - 1),
                        )
                nk = len(ks)
                k0 = ks[0]
                nc.vector.tensor_add(
                    out=v_sb[
                        :, 4 * tt + k0 : 4 * tt + k0 + nk, 2 * ghalf : 2 * ghalf + 2, 0:D
                    ],
                    in0=ps[:, 128 * (k0 % 4) : 128 * ((k0 % 4) + nk)].rearrange(
                        "p (k g d) -> p k g d", k=nk, g=2
                    ),
                    in1=bv_bc[:, 128 * ghalf : 128 * (ghalf + 1)]
                    .rearrange("p (g d) -> p g d", g=2)
                    .unsqueeze(1)
                    .broadcast_to([128, nk, 2, D]),
                )

            # phase A pre-work: everything unit 0/1 needs
            psA = qkps.tile([128, 512], F32, tag="g", name="ps_q00")
            kq_mm("q", 0, 0, psA, range(HC))
            kq_drain("q", 0, 0, psA)
            psB = qkps.tile([128, 512], F32, tag="g", name="ps_q01")
            kq_mm("q", 0, 1, psB, range(HC))
            kq_drain("q", 0, 1, psB)
            psC = qkps.tile([128, 512], F32, tag="g", name="ps_k00")
            kq_mm("k", 0, 0, psC, range(HC))
            kq_drain("k", 0, 0, psC)
            psD = qkps.tile([128, 512], F32, tag="g", name="ps_v00")
            v_group(0, 0, psD, (0, 1, 2, 3))

        # ---- phase B: attention ----
        units = [(g, qh, i) for (g, qh) in ORDER for i in range(NT)]

        # fill schedule: unit -> list of emission descriptors
        #   ("kqA", kind, jt, tt) / ("kqB", kind, jt, tt): group halves
        #   ("v2", tt, ghalf, k0): 2-tile v half-group
        #   ("c0", t) / ("c1", t): out-proj halves for token tile t
        #   ("junk",): one [128,512] junk matmul (HAM warmth)
        fills = {u: [] for u in range(len(units))}

        def place(u, *item):
            fills[u].append(item)

        def place_group(u, kind, jt, tt):
            place(u, "kqA", kind, jt, tt)
            place(u + 1, "kqB", kind, jt, tt)

        # Dense-front schedule: HAM stays at full clock only when the PE is
        # nearly saturated, so all projection fills pack into u0-39 (~93%
        # duty); the bare back half runs at minimum per-unit PE cost so a
        # throttled PE loses the least against the exp stream.
        place_group(0, "k", 0, 1)
        place(2, "v2", 1, 0, 0)
        place(3, "v2", 1, 0, 2)
        place_group(4, "k", 0, 2)
        place(6, "v2", 2, 0, 0)
        place(7, "v2", 2, 0, 2)
        place_group(8, "k", 0, 3)
        place(10, "v2", 3, 0, 0)
        place(11, "v2", 3, 0, 2)
        place_group(12, "q", 0, 2)
        place_group(14, "q", 0, 3)
        place_group(16, "q", 1, 2)
        place_group(18, "q", 1, 3)
        place_group(20, "k", 1, 0)
        place(22, "v2", 0, 1, 0)
        place(23, "v2", 0, 1, 2)
        place_group(24, "k", 1, 1)
        place(26, "v2", 1, 1, 0)
        place(27, "v2", 1, 1, 2)
        place_group(28, "k", 1, 2)
        place(30, "v2", 2, 1, 0)
        place(31, "v2", 2, 1, 2)
        place_group(32, "k", 1, 3)
        place(34, "v2", 3, 1, 0)
        place(35, "v2", 3, 1, 2)
        place_group(36, "q", 1, 0)
        place_group(38, "q", 1, 1)
        # c0 out-proj partials (need all of heads 0,1: ready ~u66)
        for t in range(NT):
            place(66 + t, "c0", t)
        # c1 finals for tokens 1024-2047 (tiles 8-15, ready ~u97) + DMA out
        for t in range(8, NT):
            place(97 + (t - 8), "c1", t)
        # junk pad: every unit with no real fill gets JUNK junk matmuls
        if JUNK:
            for u in range(16, len(units)):
                if not fills[u]:
                    for _ in range(JUNK):
                        place(u, "junk")

        with (
            tc.tile_pool(name="expp", bufs=15 if USE_DR_PV else 30) as expp,
            tc.tile_pool(name="rz", bufs=2) as rzp,
            tc.tile_pool(name="ostage", bufs=2) as ostage,
            tc.tile_pool(name="scoreps", bufs=2, space="PSUM") as score_ps,
            tc.tile_pool(name="accps", bufs=1, space="PSUM") as acc_ps,
            tc.tile_pool(name="fillps", bufs=2, space="PSUM") as fill_ps,
        ):
            pending_kq = {}
            ot_state = {}

            def emit_c0(t):
                ps = fill_ps.tile([128, 512], F32, tag="f", name=f"c0_{t}")
                nc.tensor.matmul(
                    ps,
                    lhsT=attnT[:, 0, 128 * t : 128 * (t + 1)],
                    rhs=wp_sb[:, 0, :],
                    start=True,
                    stop=True,
                )
                nc.vector.tensor_copy(out=part_sb[:, t, :], in_=ps)

            def emit_c1(t):
                ps = fill_ps.tile([128, 512], F32, tag="f", name=f"c1_{t}")
                nc.tensor.matmul(
                    ps,
                    lhsT=attnT[:, 1, 128 * t : 128 * (t + 1)],
                    rhs=wp_sb[:, 1, :],
                    start=True,
                    stop=True,
                )
                tg, k = t // 4, t % 4
                if k == 0:
                    ot_state[tg] = ostage.tile(
                        [128, 4, H], BF16, tag="ot", name=f"ot_{tg}"
                    )
                nc.vector.tensor_add(
                    out=ot_state[tg][:, k, :], in0=ps, in1=part_sb[:, t, :]
                )
                if k == 3:
                    eng = nc.sync if tg % 2 == 0 else nc.gpsimd
                    eng.dma_start(
                        out=out[512 * tg : 512 * (tg + 1), :].rearrange(
                            "(i p) j -> p i j", p=128
                        ),
                        in_=ot_state[tg],
                    )

            def emit_fill(f):
                kind = f[0]
                if kind == "kqA":
                    _, kq, jt, tt = f
                    ps = fill_ps.tile([128, 512], F32, tag="f", name=f"f{kq}{jt}{tt}")
                    pending_kq[(kq, jt, tt)] = ps
                    kq_mm(kq, jt, tt, ps, (0, 1))
                elif kind == "kqB":
                    _, kq, jt, tt = f
                    ps = pending_kq.pop((kq, jt, tt))
                    kq_mm(kq, jt, tt, ps, (2, 3))
                    kq_drain(kq, jt, tt, ps)
                elif kind == "v2":
                    _, tt, ghalf, k0 = f
                    ps = fill_ps.tile([128, 512], F32, tag="f", name=f"fv{tt}{ghalf}{k0}")
                    v_group(tt, ghalf, ps, (k0, k0 + 1))
                elif kind == "c0":
                    emit_c0(f[1])
                elif kind == "c1":
                    emit_c1(f[1])
                else:  # junk
                    ps = fill_ps.tile([128, 512], F32, tag="f", name="junk")
                    nc.tensor.matmul(
                        ps,
                        lhsT=w_sb[:, 0, 0:128],
                        rhs=xT[:, 0, 0:512],
                        start=True,
                        stop=True,
                    )

            accs = {}
            eps = {}

            def normalize(g, qh):
                # acc rows 0-63 = attn values, row 64 = Z (ones-row trick).
                # Split in 512-halves so zc/recip/broadcast/mul pipeline. The
                # Z row is copied to partition 0 first (composite DVE ops
                # mishandle nonzero input base partitions).
                jt, off = g // 2, D * (g % 2)
                a = accs[(g, qh)]
                for h in range(2):
                    hl = slice(512 * h, 512 * (h + 1))
                    zc = rzp.tile([1, 512], F32, tag="zc", name=f"zc_{g}_{qh}_{h}")
                    nc.vector.tensor_copy(out=zc, in_=a[D : D + 1, hl])
                    rz = rzp.tile([1, 512], F32, tag="rz", name=f"rz_{g}_{qh}_{h}")
                    nc.vector.reciprocal_approx_fast(out=rz, in_=zc)
                    ac = rzp.tile([D, 512], F32, tag="ac", name=f"ac_{g}_{qh}_{h}")
                    nc.vector.tensor_copy(out=ac, in_=a[0:D, hl])
                    rzb = rzp.tile([D, 512], F32, tag="rzb", name=f"rzb_{g}_{qh}_{h}")
                    nc.gpsimd.partition_broadcast(rzb, rz)
                    nc.vector.tensor_mul(
                        out=attnT[off : off + D, jt, 1024 * qh + 512 * h : 1024 * qh + 512 * (h + 1)],
                        in0=ac,
                        in1=rzb,
                    )

            npairs = NT // 2

            def do_pv_pair(idx_pair):
                # PV for key-tile pair p of block blk, fp8 DoubleRow
                blk, p = idx_pair
                g, qh = ORDER[blk]
                epp = eps.pop(idx_pair)
                for t in range(2):
                    nc.tensor.matmul(
                        accs[(g, qh)][0 : D + 1, 512 * t : 512 * (t + 1)],
                        lhsT=v_sb[:, 2 * p : 2 * p + 2, g, 0 : D + 1],
                        rhs=epp[:, 0:2, 512 * t : 512 * (t + 1)],
                        start=(p == 0),
                        stop=(p == npairs - 1),
                        perf_mode=DR,
                    )
                if p == npairs - 1:
                    normalize(g, qh)

            def do_pv_single(idx):
                pg, pqh, pi = units[idx]
                epp = eps.pop(idx)
                for t in range(2):
                    nc.tensor.matmul(
                        accs[(pg, pqh)][0 : D + 1, 512 * t : 512 * (t + 1)],
                        lhsT=v_sb[:, pi, pg, 0 : D + 1],
                        rhs=epp[:, 512 * t : 512 * (t + 1)],
                        start=(pi == 0),
                        stop=(pi == NT - 1),
                    )
                if pi == NT - 1:
                    normalize(pg, pqh)

            pv_q = []  # ready PV work items

            for idx, (g, qh, i) in enumerate(units):
                jt, off = g // 2, D * (g % 2)
                # drain one ready PV (runs 1-2 units behind the exp stream)
                if pv_q:
                    item = pv_q.pop(0)
                    if USE_DR_PV:
                        do_pv_pair(item)
                    else:
                        do_pv_single(item)
                if i == 0:
                    accs[(g, qh)] = acc_ps.tile(
                        [128, 1024], F32, tag="acc", name=f"acc_{g}_{qh}"
                    )
                kh = kT[jt][off : off + D, 128 * i : 128 * (i + 1)]
                sp = score_ps.tile([128, 1024], F32, tag="sp", name=f"sp_{idx}")
                for t in range(2):
                    nc.tensor.matmul(
                        sp[:, 512 * t : 512 * (t + 1)],
                        lhsT=kh,
                        rhs=qT[jt][
                            off : off + D,
                            1024 * qh + 512 * t : 1024 * qh + 512 * (t + 1),
                        ],
                        start=True,
                        stop=True,
                    )
                if USE_DR_PV:
                    blk, p = idx // NT, (idx % NT) // 2
                    if i % 2 == 0:
                        eps[(blk, p)] = expp.tile(
                            [128, 2, 1024], FP8, tag="ep", name=f"ep_{blk}_{p}"
                        )
                    nc.scalar.activation(eps[(blk, p)][:, i % 2, :], sp, Act.Exp)
                    if i % 2 == 1:
                        pv_q.append((blk, p))
                else:
                    ep = expp.tile([128, 1024], BF16, tag="ep", name=f"ep_{idx}")
                    nc.scalar.activation(ep, sp, Act.Exp)
                    eps[idx] = ep
                    pv_q.append(idx)
                for f in fills.get(idx, ()):
                    emit_fill(f)
            while pv_q:
                item = pv_q.pop(0)
                if USE_DR_PV:
                    do_pv_pair(item)
                else:
                    do_pv_single(item)

            # ---- tail: c=1 out-proj for tokens 0-1023 ----
            for t in range(8):
                emit_c1(t)

    nc.compile()
    return nc


_CACHE = {}


def _get_nc():
    if "nc" not in _CACHE:
        _CACHE["nc"] = build_nc()
    return _CACHE["nc"]


def make_in_maps(x, w_qkv, b_qkv, w_proj, b_proj):
    import ml_dtypes

    bf16 = ml_dtypes.bfloat16
    x = np.asarray(x, dtype=np.float32)
    w_qkv = np.asarray(w_qkv, dtype=np.float32)
    b_qkv = np.asarray(b_qkv, dtype=np.float32)
    w_proj = np.asarray(w_proj, dtype=np.float32)
    in_maps = []
    for c in range(8):
        b, hg = c // 2, c % 2
        s = 256 * hg
        w_slice = np.hstack(
            [
                w_qkv[:, s : s + 256],
                w_qkv[:, 512 + s : 512 + s + 256],
                w_qkv[:, 1024 + s : 1024 + s + 256],
            ]
        )
        b_slice = np.concatenate(
            [
                b_qkv[s : s + 256],
                b_qkv[512 + s : 512 + s + 256],
                b_qkv[1024 + s : 1024 + s + 256],
            ]
        )
        in_maps.append(
            {
                "x_in": np.ascontiguousarray(x[b].astype(bf16).T),
                "w_in": np.ascontiguousarray(w_slice.astype(bf16)),
                "b_in": np.ascontiguousarray(b_slice.astype(np.float32)),
                "wp_in": np.ascontiguousarray(w_proj[s : s + 256, :].astype(bf16)),
            }
        )
    return in_maps


def assemble(results, b_proj):
    full = np.empty((B, T, H), dtype=np.float32)
    for b in range(B):
        full[b] = (
            results[2 * b]["out"].astype(np.float32)
            + results[2 * b + 1]["out"].astype(np.float32)
            + np.asarray(b_proj, dtype=np.float32)[None, :]
        )
    return full


def kernel(x, w_qkv, b_qkv, w_proj, b_proj):
    from concourse.bass_utils import run_bass_kernel_spmd

    nc = _get_nc()
    in_maps = make_in_maps(x, w_qkv, b_qkv, w_proj, b_proj)
    res = run_bass_kernel_spmd(nc, in_maps, core_ids=list(range(8)))
    return assemble(res.results, b_proj)


# revision 13
# speedup vs baseline: 1.0130x; 1.0130x over previous
"""Trainium2 Bass kernel for an attention block (B=4, T=2048, H=512, 8 heads).

Sharding: 8 cores = 4 batches x 2 head-groups (4 heads each). Core c handles
batch c//2 and heads [4*(c%2), 4*(c%2)+4) over the FULL 2048-token context.
Each core emits a PARTIAL output projection (its 256 attn features x its
w_proj row-slice); the host sums the two partials per batch and adds b_proj.

v2 schedule, built around two measured facts:
  - ScalarE exp over [4 heads, 2048 q, 2048 k] is 128 ACTIVATEs of [128,1024]
    at ~1.09us each = ~140us. That stream is the floor; nothing else may ever
    block it.
  - The PE HAM clock gate halves the PE clock whenever the PE array has idle
    slack in its 3.4us activity window. At 1.2GHz the bf16 score+PV work
    (2048 cols/unit) exceeds the exp period (1147ns) and the exp stream
    starves. The baseline oscillated between these states and lost 60-100us.

  Fixes:
  - PV runs as fp8e4 DoubleRow over key-tile PAIRS (contraction 256/pass):
    512 PE cols/unit instead of 1024. exp output is written fp8 directly by
    the ACT (free cast); V is drained to fp8 by the DVE (free cast). Scores
    stay bf16 (K=64 contraction gains nothing from fp8).
  - Block order [g0qh0 g1qh0 g0qh1 g1qh1 g2qh1 g3qh1 g2qh0 g3qh0] over
    1024-wide query halves: heads 0,1 finish all tokens by unit 64, so the
    c=0 half of the output projection streams as PE filler from unit 66; the
    c=1 half for tokens 1024-2047 streams from unit 97 (with DMA-out); only
    tokens 0-1023's c=1 half remains for the tail.
  - All remaining QKV projections drip as deadline-scheduled fills, ~1 per
    unit, spread across the whole stream instead of front-loaded bursts.
    Optional junk matmuls pad the thin late blocks to keep the HAM warm.
  - PSUM: 2 score bufs (4 banks) + [65,1024] accumulator (2 banks, ones-row
    softmax denominator trick) + fill ping-pong (2 banks) = all 8.
"""

import sys

sys.path.insert(0, "/opt/trn_rl_repo")

from contextlib import ExitStack

import numpy as np

import concourse.bass as bass
import concourse.tile as tile
from concourse import bacc, mybir

F32 = mybir.dt.float32
BF16 = mybir.dt.bfloat16
FP8 = mybir.dt.float8e4
Act = mybir.ActivationFunctionType
DR = mybir.MatmulPerfMode.DoubleRow

B, T, H = 4, 2048, 512
HEADS = 8
D = H // HEADS  # 64
GC = 4  # heads per core
HC = H // 128  # 4 x-feature chunks
NT = T // 128  # 16 key tiles
NTT = T // 512  # 4 token 512-chunks
SCALE = float(D) ** -0.5
DP = 68  # padded v row stride (fp8 bytes): 64 d + ones + pad, %16 == 0

USE_DR_PV = True  # fp8e4 DoubleRow PV (pairs of key tiles)
JUNK = 0  # junk matmuls per unit in fill-starved units (off: measured net loss)

# block order: (g, qh) over 1024-token query halves
ORDER = [(0, 0), (1, 0), (0, 1), (1, 1), (2, 1), (3, 1), (2, 0), (3, 0)]


def build_nc():
    nc = bacc.Bacc("TRN2", target_bir_lowering=False, debug=False, num_devices=8)

    x_in = nc.dram_tensor("x_in", [H, T], BF16, kind="ExternalInput").ap()  # x.T
    w_in = nc.dram_tensor("w_in", [H, 3 * 256], BF16, kind="ExternalInput").ap()
    # bias layout: [bq*SCALE (256) | bk (256)] fp32, bv (256) fp32
    b_in = nc.dram_tensor("b_in", [3 * 256], F32, kind="ExternalInput").ap()
    wp_in = nc.dram_tensor("wp_in", [256, H], BF16, kind="ExternalInput").ap()
    out = nc.dram_tensor("out", [T, H], BF16, kind="ExternalOutput").ap()

    with tile.TileContext(nc) as tc, ExitStack() as ctx:
        per = ctx.enter_context(tc.tile_pool(name="persist", bufs=1))

        w_sb = per.tile([128, HC, 3 * 256], BF16)
        wp_sb = per.tile([128, 2, H], BF16)
        bqk_sb = per.tile([128, 4], F32)  # q bias chunks 0-1, k bias chunks 2-3
        bv_row = per.tile([1, 256], F32)
        bv_bc = per.tile([128, 256], F32)
        dummy = per.tile([1, 8], F32)
        warm = per.tile([128, 128], BF16)

        xT = per.tile([128, HC, T], BF16)
        kT = [per.tile([128, T], BF16, name=f"kT_{j}") for j in range(2)]
        qT = [per.tile([128, T], BF16, name=f"qT_{j}") for j in range(2)]
        v_dt = FP8 if USE_DR_PV else BF16
        v_sb = per.tile([128, NT, GC, DP], v_dt)
        attnT = per.tile([128, 2, T], BF16)
        part_sb = per.tile([128, NT, 512], F32)  # c=0 out-proj partials

        # ---- phase A ----
        with (
            tc.tile_pool(name="qkps", bufs=4, space="PSUM") as qkps,
            tc.tile_pool(name="warmps", bufs=1, space="PSUM") as warmps,
        ):
            # x is pre-transposed on the host. Few, big DMAs: w first on scalar
            # (the first matmuls need it), x quarters split across queues.
            def x_dma(eng, clo, tlo, thi):
                eng.dma_start(
                    out=xT[:, clo : clo + 2, tlo:thi],
                    in_=x_in[128 * clo : 128 * (clo + 2), tlo:thi].rearrange(
                        "(c p) t -> p c t", p=128
                    ),
                )

            nc.scalar.dma_start(
                out=w_sb[:, :, 0:512],
                in_=w_in[:, 0:512].rearrange("(c p) j -> p c j", p=128),
            )
            nc.scalar.dma_start(
                out=w_sb[:, :, 512:768],
                in_=w_in[:, 512:768].rearrange("(c p) j -> p c j", p=128),
            )
            nc.gpsimd.dma_start(
                out=bqk_sb, in_=b_in[0:512].rearrange("(c p) -> p c", p=128)
            )
            nc.gpsimd.dma_start(out=bv_row, in_=b_in[512:768].unsqueeze(0))
            x_dma(nc.sync, 0, 0, 512)
            x_dma(nc.gpsimd, 2, 0, 512)
            x_dma(nc.sync, 0, 512, 1024)
            x_dma(nc.gpsimd, 2, 512, 1024)
            x_dma(nc.sync, 2, 1024, 2048)
            x_dma(nc.scalar, 0, 1024, 2048)
            nc.scalar.dma_start(
                out=wp_sb, in_=wp_in.rearrange("(c p) j -> p c j", p=128)
            )
            # junk matmuls warm the PE clock gate (HAM) during the DMA wait so
            # the first real projections run at 2.4 GHz
            nc.vector.memset(warm, 0.0)
            wps = warmps.tile([128, 128], F32, tag="w", name="warmps")
            for _ in range(48):
                nc.tensor.matmul(wps, lhsT=warm, rhs=warm, start=True, stop=True)
            # preload the exp table while DMAs stream
            nc.gpsimd.memset(dummy, 0.0)
            nc.scalar.activation(dummy, dummy, Act.Exp)
            nc.gpsimd.partition_broadcast(bv_bc, bv_row)
            nc.vector.memset(v_sb[:, :, :, D : D + 1], 1.0)

            # K/Q projection group: 4 matmuls + a drain; emitted in halves
            # (c-chunk pairs) so fills spread smoothly.
            def kq_mm(kind, jt, tt, ps, cs):
                col0 = 256 + 128 * jt if kind == "k" else 128 * jt
                for c in cs:
                    nc.tensor.matmul(
                        ps[:, 0:512],
                        lhsT=w_sb[:, c, col0 : col0 + 128],
                        rhs=xT[:, c, 512 * tt : 512 * (tt + 1)],
                        start=(c == 0),
                        stop=(c == HC - 1),
                    )

            def kq_drain(kind, jt, tt, ps):
                if kind == "k":
                    nc.vector.tensor_scalar(
                        out=kT[jt][:, 512 * tt : 512 * (tt + 1)],
                        in0=ps[:, 0:512],
                        scalar1=bqk_sb[:, 2 + jt : 3 + jt],
                        scalar2=None,
                        op0=mybir.AluOpType.add,
                    )
                else:
                    nc.vector.tensor_scalar(
                        out=qT[jt][:, 512 * tt : 512 * (tt + 1)],
                        in0=ps[:, 0:512],
                        scalar1=bqk_sb[:, jt : jt + 1],
                        scalar2=SCALE,
                        op0=mybir.AluOpType.add,
                        op1=mybir.AluOpType.mult,
                    )

            # V for head range [2*ghalf, 2*ghalf+2) over `ks` 128-token tiles
            # of 512-chunk tt: 4 matmuls per tile, one strided drain.
            def v_group(tt, ghalf, ps, ks):
                for k in ks:
                    i = 4 * tt + k
                    for c in range(HC):
                        nc.tensor.matmul(
                            ps[:, 128 * (k % 4) : 128 * (k % 4) + 128],
                            lhsT=xT[:, c, 128 * i : 128 * (i + 1)],
                            rhs=w_sb[:, c, 512 + 128 * ghalf : 512 + 128 * (ghalf + 1)],
                            start=(c == 0),
                            stop=(c == HC - 1),
                        )
                nk = len(ks)
                k0 = ks[0]
                nc.vector.tensor_add(
                    out=v_sb[
                        :, 4 * tt + k0 : 4 * tt + k0 + nk, 2 * ghalf : 2 * ghalf + 2, 0:D
                    ],
                    in0=ps[:, 128 * (k0 % 4) : 128 * ((k0 % 4) + nk)].rearrange(
                        "p (k g d) -> p k g d", k=nk, g=2
                    ),
                    in1=bv_bc[:, 128 * ghalf : 128 * (ghalf + 1)]
                    .rearrange("p (g d) -> p g d", g=2)
                    .unsqueeze(1)
                    .broadcast_to([128, nk, 2, D]),
                )

            # phase A pre-work: everything unit 0/1 needs
            psA = qkps.tile([128, 512], F32, tag="g", name="ps_q00")
            kq_mm("q", 0, 0, psA, range(HC))
            kq_drain("q", 0, 0, psA)
            psB = qkps.tile([128, 512], F32, tag="g", name="ps_q01")
            kq_mm("q", 0, 1, psB, range(HC))
            kq_drain("q", 0, 1, psB)
            psC = qkps.tile([128, 512], F32, tag="g", name="ps_k00")
            kq_mm("k", 0, 0, psC, range(HC))
            kq_drain("k", 0, 0, psC)
            psD = qkps.tile([128, 512], F32, tag="g", name="ps_v00")
            v_group(0, 0, psD, (0, 1, 2, 3))

        # ---- phase B: attention ----
        units = [(g, qh, i) for (g, qh) in ORDER for i in range(NT)]

        # fill schedule: unit -> list of emission descriptors
        #   ("kqA", kind, jt, tt) / ("kqB", kind, jt, tt): group halves
        #   ("v2", tt, ghalf, k0): 2-tile v half-group
        #   ("c0", t) / ("c1", t): out-proj halves for token tile t
        #   ("junk",): one [128,512] junk matmul (HAM warmth)
        fills = {u: [] for u in range(len(units))}

        def place(u, *item):
            fills[u].append(item)

        def place_group(u, kind, jt, tt):
            place(u, "kqA", kind, jt, tt)
            place(u + 1, "kqB", kind, jt, tt)

        # HAM recovery-burst schedule. Measured behavior: once the clock gate
        # drops to 4/8 it only recovers after a ~3.4us window of STALL-FREE
        # PE saturation; the steady score/PV stream micro-stalls on the exp
        # pipeline and never recovers. So every 8 units carries one ~4096-
        # cycle back-to-back burst (a double projection group where real work
        # remains, junk matmuls after) sized to be absorbed by the 2-deep
        # score pipeline when the clock is already at 8/8.
        # B0: forced JIT fills (kT0/v-gh0 stream just ahead of the scores).
        place_group(0, "k", 0, 1)
        place(2, "v2", 1, 0, 0)
        place(3, "v2", 1, 0, 2)
        place_group(4, "k", 0, 2)
        place(6, "v2", 2, 0, 0)
        place(7, "v2", 2, 0, 2)
        place_group(8, "k", 0, 3)
        place(10, "v2", 3, 0, 0)
        place(11, "v2", 3, 0, 2)

        def burst2(u, spec1, spec2):
            # spec: ("q"/"k", jt, tt) or ("v", tt, ghalf)
            for kind, a1, a2 in (spec1, spec2):
                if kind == "v":
                    place(u, "v2", a1, a2, 0)
                    place(u, "v2", a1, a2, 2)
                else:
                    place(u, "kqA", kind, a1, a2)
                    place(u, "kqB", kind, a1, a2)

        burst2(16, ("q", 0, 2), ("q", 0, 3))
        burst2(24, ("q", 1, 2), ("q", 1, 3))
        burst2(32, ("k", 1, 0), ("v", 0, 1))
        burst2(40, ("k", 1, 1), ("v", 1, 1))
        burst2(48, ("k", 1, 2), ("v", 2, 1))
        burst2(56, ("k", 1, 3), ("v", 3, 1))
        burst2(64, ("q", 1, 0), ("q", 1, 1))
        for u in (72, 80, 88, 96, 104, 112, 120):
            for _ in range(8):
                place(u, "junk")
        # c0 out-proj partials (need all of heads 0,1: ready ~u66)
        for t in range(NT):
            place(66 + t, "c0", t)
        # c1 finals for tokens 1024-2047 (tiles 8-15, ready ~u97) + DMA out
        for t in range(8, NT):
            place(97 + (t - 8), "c1", t)

        with (
            tc.tile_pool(name="expp", bufs=15 if USE_DR_PV else 30) as expp,
            tc.tile_pool(name="rz", bufs=2) as rzp,
            tc.tile_pool(name="ostage", bufs=2) as ostage,
            tc.tile_pool(name="scoreps", bufs=2, space="PSUM") as score_ps,
            tc.tile_pool(name="accps", bufs=1, space="PSUM") as acc_ps,
            tc.tile_pool(name="fillps", bufs=2, space="PSUM") as fill_ps,
        ):
            pending_kq = {}
            ot_state = {}

            def emit_c0(t):
                ps = fill_ps.tile([128, 512], F32, tag="f", name=f"c0_{t}")
                nc.tensor.matmul(
                    ps,
                    lhsT=attnT[:, 0, 128 * t : 128 * (t + 1)],
                    rhs=wp_sb[:, 0, :],
                    start=True,
                    stop=True,
                )
                nc.vector.tensor_copy(out=part_sb[:, t, :], in_=ps)

            def emit_c1(t):
                ps = fill_ps.tile([128, 512], F32, tag="f", name=f"c1_{t}")
                nc.tensor.matmul(
                    ps,
                    lhsT=attnT[:, 1, 128 * t : 128 * (t + 1)],
                    rhs=wp_sb[:, 1, :],
                    start=True,
                    stop=True,
                )
                tg, k = t // 4, t % 4
                if k == 0:
                    ot_state[tg] = ostage.tile(
                        [128, 4, H], BF16, tag="ot", name=f"ot_{tg}"
                    )
                nc.vector.tensor_add(
                    out=ot_state[tg][:, k, :], in0=ps, in1=part_sb[:, t, :]
                )
                if k == 3:
                    eng = {0: nc.sync, 1: nc.scalar, 2: nc.sync, 3: nc.gpsimd}[tg]
                    eng.dma_start(
                        out=out[512 * tg : 512 * (tg + 1), :].rearrange(
                            "(i p) j -> p i j", p=128
                        ),
                        in_=ot_state[tg],
                    )

            def emit_fill(f):
                kind = f[0]
                if kind == "kqA":
                    _, kq, jt, tt = f
                    ps = fill_ps.tile([128, 512], F32, tag="f", name=f"f{kq}{jt}{tt}")
                    pending_kq[(kq, jt, tt)] = ps
                    kq_mm(kq, jt, tt, ps, (0, 1))
                elif kind == "kqB":
                    _, kq, jt, tt = f
                    ps = pending_kq.pop((kq, jt, tt))
                    kq_mm(kq, jt, tt, ps, (2, 3))
                    kq_drain(kq, jt, tt, ps)
                elif kind == "v2":
                    _, tt, ghalf, k0 = f
                    ps = fill_ps.tile([128, 512], F32, tag="f", name=f"fv{tt}{ghalf}{k0}")
                    v_group(tt, ghalf, ps, (k0, k0 + 1))
                elif kind == "c0":
                    emit_c0(f[1])
                elif kind == "c1":
                    emit_c1(f[1])
                else:  # junk
                    ps = fill_ps.tile([128, 512], F32, tag="f", name="junk")
                    nc.tensor.matmul(
                        ps,
                        lhsT=w_sb[:, 0, 0:128],
                        rhs=xT[:, 0, 0:512],
                        start=True,
                        stop=True,
                    )

            accs = {}
            eps = {}

            def normalize(g, qh):
                # acc rows 0-63 = attn values, row 64 = Z (ones-row trick).
                # Split in 512-halves so zc/recip/broadcast/mul pipeline. The
                # Z row is copied to partition 0 first (composite DVE ops
                # mishandle nonzero input base partitions).
                jt, off = g // 2, D * (g % 2)
                a = accs[(g, qh)]
                for h in range(2):
                    hl = slice(512 * h, 512 * (h + 1))
                    zc = rzp.tile([1, 512], F32, tag="zc", name=f"zc_{g}_{qh}_{h}")
                    nc.vector.tensor_copy(out=zc, in_=a[D : D + 1, hl])
                    rz = rzp.tile([1, 512], F32, tag="rz", name=f"rz_{g}_{qh}_{h}")
                    nc.vector.reciprocal_approx_fast(out=rz, in_=zc)
                    ac = rzp.tile([D, 512], F32, tag="ac", name=f"ac_{g}_{qh}_{h}")
                    nc.vector.tensor_copy(out=ac, in_=a[0:D, hl])
                    rzb = rzp.tile([D, 512], F32, tag="rzb", name=f"rzb_{g}_{qh}_{h}")
                    nc.gpsimd.partition_broadcast(rzb, rz)
                    nc.vector.tensor_mul(
                        out=attnT[off : off + D, jt, 1024 * qh + 512 * h : 1024 * qh + 512 * (h + 1)],
                        in0=ac,
                        in1=rzb,
                    )

            npairs = NT // 2

            def do_pv_pair(idx_pair):
                # PV for key-tile pair p of block blk, fp8 DoubleRow
                blk, p = idx_pair
                g, qh = ORDER[blk]
                epp = eps.pop(idx_pair)
                for t in range(2):
                    nc.tensor.matmul(
                        accs[(g, qh)][0 : D + 1, 512 * t : 512 * (t + 1)],
                        lhsT=v_sb[:, 2 * p : 2 * p + 2, g, 0 : D + 1],
                        rhs=epp[:, 0:2, 512 * t : 512 * (t + 1)],
                        start=(p == 0),
                        stop=(p == npairs - 1),
                        perf_mode=DR,
                    )
                if p == npairs - 1:
                    normalize(g, qh)

            def do_pv_single(idx):
                pg, pqh, pi = units[idx]
                epp = eps.pop(idx)
                for t in range(2):
                    nc.tensor.matmul(
                        accs[(pg, pqh)][0 : D + 1, 512 * t : 512 * (t + 1)],
                        lhsT=v_sb[:, pi, pg, 0 : D + 1],
                        rhs=epp[:, 512 * t : 512 * (t + 1)],
                        start=(pi == 0),
                        stop=(pi == NT - 1),
                    )
                if pi == NT - 1:
                    normalize(pg, pqh)

            pv_q = []  # ready PV work items

            for idx, (g, qh, i) in enumerate(units):
                jt, off = g // 2, D * (g % 2)
                # drain one ready PV (runs 1-2 units behind the exp stream)
                if pv_q:
                    item = pv_q.pop(0)
                    if USE_DR_PV:
                        do_pv_pair(item)
                    else:
                        do_pv_single(item)
                if i == 0:
                    accs[(g, qh)] = acc_ps.tile(
                        [128, 1024], F32, tag="acc", name=f"acc_{g}_{qh}"
                    )
                kh = kT[jt][off : off + D, 128 * i : 128 * (i + 1)]
                sp = score_ps.tile([128, 1024], F32, tag="sp", name=f"sp_{idx}")
                for t in range(2):
                    nc.tensor.matmul(
                        sp[:, 512 * t : 512 * (t + 1)],
                        lhsT=kh,
                        rhs=qT[jt][
                            off : off + D,
                            1024 * qh + 512 * t : 1024 * qh + 512 * (t + 1),
                        ],
                        start=True,
                        stop=True,
                    )
                if USE_DR_PV:
                    blk, p = idx // NT, (idx % NT) // 2
                    if i % 2 == 0:
                        eps[(blk, p)] = expp.tile(
                            [128, 2, 1024], FP8, tag="ep", name=f"ep_{blk}_{p}"
                        )
                    nc.scalar.activation(eps[(blk, p)][:, i % 2, :], sp, Act.Exp)
                    if i % 2 == 1:
                        pv_q.append((blk, p))
                else:
                    ep = expp.tile([128, 1024], BF16, tag="ep", name=f"ep_{idx}")
                    nc.scalar.activation(ep, sp, Act.Exp)
                    eps[idx] = ep
                    pv_q.append(idx)
                for f in fills.get(idx, ()):
                    emit_fill(f)
            while pv_q:
                item = pv_q.pop(0)
                if USE_DR_PV:
                    do_pv_pair(item)
                else:
                    do_pv_single(item)

            # ---- tail: c=1 out-proj for tokens 0-1023 ----
            for t in range(8):
                emit_c1(t)

    nc.compile()
    return nc


_CACHE = {}


def _get_nc():
    if "nc" not in _CACHE:
        _CACHE["nc"] = build_nc()
    return _CACHE["nc"]


def make_in_maps(x, w_qkv, b_qkv, w_proj, b_proj):
    import ml_dtypes

    bf16 = ml_dtypes.bfloat16
    x = np.asarray(x, dtype=np.float32)
    w_qkv = np.asarray(w_qkv, dtype=np.float32)
    b_qkv = np.asarray(b_qkv, dtype=np.float32)
    w_proj = np.asarray(w_proj, dtype=np.float32)
    in_maps = []
    for c in range(8):
        b, hg = c // 2, c % 2
        s = 256 * hg
        w_slice = np.hstack(
            [
                w_qkv[:, s : s + 256],
                w_qkv[:, 512 + s : 512 + s + 256],
                w_qkv[:, 1024 + s : 1024 + s + 256],
            ]
        )
        b_slice = np.concatenate(
            [
                b_qkv[s : s + 256],
                b_qkv[512 + s : 512 + s + 256],
                b_qkv[1024 + s : 1024 + s + 256],
            ]
        )
        in_maps.append(
            {
                "x_in": np.ascontiguousarray(x[b].astype(bf16).T),
                "w_in": np.ascontiguousarray(w_slice.astype(bf16)),
                "b_in": np.ascontiguousarray(b_slice.astype(np.float32)),
                "wp_in": np.ascontiguousarray(w_proj[s : s + 256, :].astype(bf16)),
            }
        )
    return in_maps


def assemble(results, b_proj):
    full = np.empty((B, T, H), dtype=np.float32)
    for b in range(B):
        full[b] = (
            results[2 * b]["out"].astype(np.float32)
            + results[2 * b + 1]["out"].astype(np.float32)
            + np.asarray(b_proj, dtype=np.float32)[None, :]
        )
    return full


def kernel(x, w_qkv, b_qkv, w_proj, b_proj):
    from concourse.bass_utils import run_bass_kernel_spmd

    nc = _get_nc()
    in_maps = make_in_maps(x, w_qkv, b_qkv, w_proj, b_proj)
    res = run_bass_kernel_spmd(nc, in_maps, core_ids=list(range(8)))
    return assemble(res.results, b_proj)


# revision 14
# speedup vs baseline: 1.0144x; 1.0014x over previous
"""Trainium2 Bass kernel for an attention block (B=4, T=2048, H=512, 8 heads).

Sharding: 8 cores = 4 batches x 2 head-groups (4 heads each). Core c handles
batch c//2 and heads [4*(c%2), 4*(c%2)+4) over the FULL 2048-token context.
Each core emits a PARTIAL output projection (its 256 attn features x its
w_proj row-slice); the host sums the two partials per batch and adds b_proj.

v2 schedule, built around two measured facts:
  - ScalarE exp over [4 heads, 2048 q, 2048 k] is 128 ACTIVATEs of [128,1024]
    at ~1.09us each = ~140us. That stream is the floor; nothing else may ever
    block it.
  - The PE HAM clock gate halves the PE clock whenever the PE array has idle
    slack in its 3.4us activity window. At 1.2GHz the bf16 score+PV work
    (2048 cols/unit) exceeds the exp period (1147ns) and the exp stream
    starves. The baseline oscillated between these states and lost 60-100us.

  Fixes:
  - PV runs as fp8e4 DoubleRow over key-tile PAIRS (contraction 256/pass):
    512 PE cols/unit instead of 1024. exp output is written fp8 directly by
    the ACT (free cast); V is drained to fp8 by the DVE (free cast). Scores
    stay bf16 (K=64 contraction gains nothing from fp8).
  - Block order [g0qh0 g1qh0 g0qh1 g1qh1 g2qh1 g3qh1 g2qh0 g3qh0] over
    1024-wide query halves: heads 0,1 finish all tokens by unit 64, so the
    c=0 half of the output projection streams as PE filler from unit 66; the
    c=1 half for tokens 1024-2047 streams from unit 97 (with DMA-out); only
    tokens 0-1023's c=1 half remains for the tail.
  - All remaining QKV projections drip as deadline-scheduled fills, ~1 per
    unit, spread across the whole stream instead of front-loaded bursts.
    Optional junk matmuls pad the thin late blocks to keep the HAM warm.
  - PSUM: 2 score bufs (4 banks) + [65,1024] accumulator (2 banks, ones-row
    softmax denominator trick) + fill ping-pong (2 banks) = all 8.
"""

import sys

sys.path.insert(0, "/opt/trn_rl_repo")

from contextlib import ExitStack

import numpy as np

import concourse.bass as bass
import concourse.tile as tile
from concourse import bacc, mybir

F32 = mybir.dt.float32
BF16 = mybir.dt.bfloat16
FP8 = mybir.dt.float8e4
Act = mybir.ActivationFunctionType
DR = mybir.MatmulPerfMode.DoubleRow

B, T, H = 4, 2048, 512
HEADS = 8
D = H // HEADS  # 64
GC = 4  # heads per core
HC = H // 128  # 4 x-feature chunks
NT = T // 128  # 16 key tiles
NTT = T // 512  # 4 token 512-chunks
SCALE = float(D) ** -0.5
DP = 68  # padded v row stride (fp8 bytes): 64 d + ones + pad, %16 == 0

USE_DR_PV = True  # fp8e4 DoubleRow PV (pairs of key tiles)
JUNK = 0  # junk matmuls per unit in fill-starved units (off: measured net loss)

# block order: (g, qh) over 1024-token query halves
ORDER = [(0, 0), (1, 0), (0, 1), (1, 1), (2, 1), (3, 1), (2, 0), (3, 0)]


def build_nc():
    nc = bacc.Bacc("TRN2", target_bir_lowering=False, debug=False, num_devices=8)

    x_in = nc.dram_tensor("x_in", [H, T], BF16, kind="ExternalInput").ap()  # x.T
    w_in = nc.dram_tensor("w_in", [H, 3 * 256], BF16, kind="ExternalInput").ap()
    # bias layout: [bq*SCALE (256) | bk (256)] fp32, bv (256) fp32
    b_in = nc.dram_tensor("b_in", [3 * 256], F32, kind="ExternalInput").ap()
    wp_in = nc.dram_tensor("wp_in", [256, H], BF16, kind="ExternalInput").ap()
    out = nc.dram_tensor("out", [T, H], BF16, kind="ExternalOutput").ap()

    with tile.TileContext(nc) as tc, ExitStack() as ctx:
        per = ctx.enter_context(tc.tile_pool(name="persist", bufs=1))

        w_sb = per.tile([128, HC, 3 * 256], BF16)
        wp_sb = per.tile([128, 2, H], BF16)
        bqk_sb = per.tile([128, 4], F32)  # q bias chunks 0-1, k bias chunks 2-3
        bv_row = per.tile([1, 256], F32)
        bv_bc = per.tile([128, 256], F32)
        dummy = per.tile([1, 8], F32)
        warm = per.tile([128, 128], BF16)

        xT = per.tile([128, HC, T], BF16)
        kT = [per.tile([128, T], BF16, name=f"kT_{j}") for j in range(2)]
        qT = [per.tile([128, T], BF16, name=f"qT_{j}") for j in range(2)]
        v_dt = FP8 if USE_DR_PV else BF16
        v_sb = per.tile([128, NT, GC, DP], v_dt)
        attnT = per.tile([128, 2, T], BF16)
        part_sb = per.tile([128, NT, 512], F32)  # c=0 out-proj partials

        # ---- phase A ----
        with (
            tc.tile_pool(name="qkps", bufs=4, space="PSUM") as qkps,
            tc.tile_pool(name="warmps", bufs=1, space="PSUM") as warmps,
        ):
            # x is pre-transposed on the host. Few, big DMAs: w first on scalar
            # (the first matmuls need it), x quarters split across queues.
            def x_dma(eng, clo, tlo, thi):
                eng.dma_start(
                    out=xT[:, clo : clo + 2, tlo:thi],
                    in_=x_in[128 * clo : 128 * (clo + 2), tlo:thi].rearrange(
                        "(c p) t -> p c t", p=128
                    ),
                )

            nc.scalar.dma_start(
                out=w_sb[:, :, 0:512],
                in_=w_in[:, 0:512].rearrange("(c p) j -> p c j", p=128),
            )
            nc.scalar.dma_start(
                out=w_sb[:, :, 512:768],
                in_=w_in[:, 512:768].rearrange("(c p) j -> p c j", p=128),
            )
            nc.gpsimd.dma_start(
                out=bqk_sb, in_=b_in[0:512].rearrange("(c p) -> p c", p=128)
            )
            nc.gpsimd.dma_start(out=bv_row, in_=b_in[512:768].unsqueeze(0))
            x_dma(nc.sync, 0, 0, 512)
            x_dma(nc.gpsimd, 2, 0, 512)
            x_dma(nc.sync, 0, 512, 1024)
            x_dma(nc.gpsimd, 2, 512, 1024)
            x_dma(nc.sync, 2, 1024, 2048)
            x_dma(nc.scalar, 0, 1024, 2048)
            nc.scalar.dma_start(
                out=wp_sb, in_=wp_in.rearrange("(c p) j -> p c j", p=128)
            )
            # junk matmuls warm the PE clock gate (HAM) during the DMA wait so
            # the first real projections run at 2.4 GHz
            nc.vector.memset(warm, 0.0)
            wps = warmps.tile([128, 128], F32, tag="w", name="warmps")
            for _ in range(48):
                nc.tensor.matmul(wps, lhsT=warm, rhs=warm, start=True, stop=True)
            # preload the exp table while DMAs stream
            nc.gpsimd.memset(dummy, 0.0)
            nc.scalar.activation(dummy, dummy, Act.Exp)
            nc.gpsimd.partition_broadcast(bv_bc, bv_row)
            nc.vector.memset(v_sb[:, :, :, D : D + 1], 1.0)

            # K/Q projection group: 4 matmuls + a drain; emitted in halves
            # (c-chunk pairs) so fills spread smoothly.
            def kq_mm(kind, jt, tt, ps, cs):
                col0 = 256 + 128 * jt if kind == "k" else 128 * jt
                for c in cs:
                    nc.tensor.matmul(
                        ps[:, 0:512],
                        lhsT=w_sb[:, c, col0 : col0 + 128],
                        rhs=xT[:, c, 512 * tt : 512 * (tt + 1)],
                        start=(c == 0),
                        stop=(c == HC - 1),
                    )

            def kq_drain(kind, jt, tt, ps):
                if kind == "k":
                    nc.vector.tensor_scalar(
                        out=kT[jt][:, 512 * tt : 512 * (tt + 1)],
                        in0=ps[:, 0:512],
                        scalar1=bqk_sb[:, 2 + jt : 3 + jt],
                        scalar2=None,
                        op0=mybir.AluOpType.add,
                    )
                else:
                    nc.vector.tensor_scalar(
                        out=qT[jt][:, 512 * tt : 512 * (tt + 1)],
                        in0=ps[:, 0:512],
                        scalar1=bqk_sb[:, jt : jt + 1],
                        scalar2=SCALE,
                        op0=mybir.AluOpType.add,
                        op1=mybir.AluOpType.mult,
                    )

            # V for head range [2*ghalf, 2*ghalf+2) over `ks` 128-token tiles
            # of 512-chunk tt: 4 matmuls per tile, one strided drain.
            def v_group(tt, ghalf, ps, ks):
                for k in ks:
                    i = 4 * tt + k
                    for c in range(HC):
                        nc.tensor.matmul(
                            ps[:, 128 * (k % 4) : 128 * (k % 4) + 128],
                            lhsT=xT[:, c, 128 * i : 128 * (i + 1)],
                            rhs=w_sb[:, c, 512 + 128 * ghalf : 512 + 128 * (ghalf + 1)],
                            start=(c == 0),
                            stop=(c == HC - 1),
                        )
                nk = len(ks)
                k0 = ks[0]
                nc.vector.tensor_add(
                    out=v_sb[
                        :, 4 * tt + k0 : 4 * tt + k0 + nk, 2 * ghalf : 2 * ghalf + 2, 0:D
                    ],
                    in0=ps[:, 128 * (k0 % 4) : 128 * ((k0 % 4) + nk)].rearrange(
                        "p (k g d) -> p k g d", k=nk, g=2
                    ),
                    in1=bv_bc[:, 128 * ghalf : 128 * (ghalf + 1)]
                    .rearrange("p (g d) -> p g d", g=2)
                    .unsqueeze(1)
                    .broadcast_to([128, nk, 2, D]),
                )

            # phase A pre-work: everything unit 0/1 needs
            psA = qkps.tile([128, 512], F32, tag="g", name="ps_q00")
            kq_mm("q", 0, 0, psA, range(HC))
            kq_drain("q", 0, 0, psA)
            psB = qkps.tile([128, 512], F32, tag="g", name="ps_q01")
            kq_mm("q", 0, 1, psB, range(HC))
            kq_drain("q", 0, 1, psB)
            psC = qkps.tile([128, 512], F32, tag="g", name="ps_k00")
            kq_mm("k", 0, 0, psC, range(HC))
            kq_drain("k", 0, 0, psC)
            psD = qkps.tile([128, 512], F32, tag="g", name="ps_v00")
            v_group(0, 0, psD, (0, 1, 2, 3))

        # ---- phase B: attention ----
        units = [(g, qh, i) for (g, qh) in ORDER for i in range(NT)]

        # fill schedule: unit -> list of emission descriptors
        #   ("kqA", kind, jt, tt) / ("kqB", kind, jt, tt): group halves
        #   ("v2", tt, ghalf, k0): 2-tile v half-group
        #   ("c0", t) / ("c1", t): out-proj halves for token tile t
        #   ("junk",): one [128,512] junk matmul (HAM warmth)
        fills = {u: [] for u in range(len(units))}

        def place(u, *item):
            fills[u].append(item)

        def place_group(u, kind, jt, tt):
            place(u, "kqA", kind, jt, tt)
            place(u + 1, "kqB", kind, jt, tt)

        # HAM recovery-burst schedule. Measured behavior: once the clock gate
        # drops to 4/8 it only recovers after a ~3.4us window of STALL-FREE
        # PE saturation; the steady score/PV stream micro-stalls on the exp
        # pipeline and never recovers. So every 8 units carries one ~4096-
        # cycle back-to-back burst (a double projection group where real work
        # remains, junk matmuls after) sized to be absorbed by the 2-deep
        # score pipeline when the clock is already at 8/8.
        # B0: forced JIT fills (kT0/v-gh0 stream just ahead of the scores).
        place_group(0, "k", 0, 1)
        place(2, "v2", 1, 0, 0)
        place(3, "v2", 1, 0, 2)
        place_group(4, "k", 0, 2)
        place(6, "v2", 2, 0, 0)
        place(7, "v2", 2, 0, 2)
        place_group(8, "k", 0, 3)
        place(10, "v2", 3, 0, 0)
        place(11, "v2", 3, 0, 2)

        def burst2(u, spec1, spec2):
            # spec: ("q"/"k", jt, tt) or ("v", tt, ghalf)
            for kind, a1, a2 in (spec1, spec2):
                if kind == "v":
                    place(u, "v2", a1, a2, 0)
                    place(u, "v2", a1, a2, 2)
                else:
                    place(u, "kqA", kind, a1, a2)
                    place(u, "kqB", kind, a1, a2)

        burst2(16, ("q", 0, 2), ("q", 0, 3))
        burst2(24, ("q", 1, 2), ("q", 1, 3))
        burst2(32, ("k", 1, 0), ("v", 0, 1))
        burst2(40, ("k", 1, 1), ("v", 1, 1))
        burst2(48, ("k", 1, 2), ("v", 2, 1))
        burst2(56, ("k", 1, 3), ("v", 3, 1))
        burst2(64, ("q", 1, 0), ("q", 1, 1))
        burst_units = (70, 76, 82, 88, 94, 100, 106, 112, 118, 124)
        for u in burst_units:
            for _ in range(10):
                place(u, "junk")

        def place_avoiding(u0, *item):
            u = u0
            while u in burst_units:
                u += 1
            place(u, *item)

        # c0 out-proj partials (need all of heads 0,1: ready ~u66)
        for t in range(NT):
            place_avoiding(66 + t, "c0", t)
        # c1 finals for tokens 1024-2047 (tiles 8-15, ready ~u97) + DMA out
        for t in range(8, NT):
            place_avoiding(97 + (t - 8), "c1", t)

        with (
            tc.tile_pool(name="expp", bufs=15 if USE_DR_PV else 30) as expp,
            tc.tile_pool(name="rz", bufs=2) as rzp,
            tc.tile_pool(name="ostage", bufs=2) as ostage,
            tc.tile_pool(name="scoreps", bufs=2, space="PSUM") as score_ps,
            tc.tile_pool(name="accps", bufs=1, space="PSUM") as acc_ps,
            tc.tile_pool(name="fillps", bufs=2, space="PSUM") as fill_ps,
        ):
            pending_kq = {}
            ot_state = {}

            def emit_c0(t):
                ps = fill_ps.tile([128, 512], F32, tag="f", name=f"c0_{t}")
                nc.tensor.matmul(
                    ps,
                    lhsT=attnT[:, 0, 128 * t : 128 * (t + 1)],
                    rhs=wp_sb[:, 0, :],
                    start=True,
                    stop=True,
                )
                nc.vector.tensor_copy(out=part_sb[:, t, :], in_=ps)

            tail_sp = {}

            def emit_c1(t, tail=False):
                if tail:
                    # scores pool is idle now; two [128,1024] tiles give a
                    # 4-deep psum rotation so the adds never stall the PE
                    if t % 2 == 0:
                        tail_sp[t // 2] = score_ps.tile(
                            [128, 1024], F32, tag="sp", name=f"c1t_{t // 2}"
                        )
                    ps = tail_sp[t // 2][:, 512 * (t % 2) : 512 * (t % 2) + 512]
                else:
                    ps = fill_ps.tile([128, 512], F32, tag="f", name=f"c1_{t}")
                nc.tensor.matmul(
                    ps,
                    lhsT=attnT[:, 1, 128 * t : 128 * (t + 1)],
                    rhs=wp_sb[:, 1, :],
                    start=True,
                    stop=True,
                )
                tg, k = t // 4, t % 4
                if k == 0:
                    ot_state[tg] = ostage.tile(
                        [128, 4, H], BF16, tag="ot", name=f"ot_{tg}"
                    )
                nc.vector.tensor_add(
                    out=ot_state[tg][:, k, :], in0=ps, in1=part_sb[:, t, :]
                )
                if k == 3:
                    eng = {0: nc.sync, 1: nc.scalar, 2: nc.sync, 3: nc.gpsimd}[tg]
                    eng.dma_start(
                        out=out[512 * tg : 512 * (tg + 1), :].rearrange(
                            "(i p) j -> p i j", p=128
                        ),
                        in_=ot_state[tg],
                    )

            def emit_fill(f):
                kind = f[0]
                if kind == "kqA":
                    _, kq, jt, tt = f
                    ps = fill_ps.tile([128, 512], F32, tag="f", name=f"f{kq}{jt}{tt}")
                    pending_kq[(kq, jt, tt)] = ps
                    kq_mm(kq, jt, tt, ps, (0, 1))
                elif kind == "kqB":
                    _, kq, jt, tt = f
                    ps = pending_kq.pop((kq, jt, tt))
                    kq_mm(kq, jt, tt, ps, (2, 3))
                    kq_drain(kq, jt, tt, ps)
                elif kind == "v2":
                    _, tt, ghalf, k0 = f
                    ps = fill_ps.tile([128, 512], F32, tag="f", name=f"fv{tt}{ghalf}{k0}")
                    v_group(tt, ghalf, ps, (k0, k0 + 1))
                elif kind == "c0":
                    emit_c0(f[1])
                elif kind == "c1":
                    emit_c1(f[1])
                else:  # junk
                    ps = fill_ps.tile([128, 512], F32, tag="f", name="junk")
                    nc.tensor.matmul(
                        ps,
                        lhsT=w_sb[:, 0, 0:128],
                        rhs=xT[:, 0, 0:512],
                        start=True,
                        stop=True,
                    )

            accs = {}
            eps = {}

            def normalize(g, qh):
                # acc rows 0-63 = attn values, row 64 = Z (ones-row trick).
                # Split in 512-halves so zc/recip/broadcast/mul pipeline. The
                # Z row is copied to partition 0 first (composite DVE ops
                # mishandle nonzero input base partitions).
                jt, off = g // 2, D * (g % 2)
                a = accs[(g, qh)]
                for h in range(2):
                    hl = slice(512 * h, 512 * (h + 1))
                    zc = rzp.tile([1, 512], F32, tag="zc", name=f"zc_{g}_{qh}_{h}")
                    nc.vector.tensor_copy(out=zc, in_=a[D : D + 1, hl])
                    rz = rzp.tile([1, 512], F32, tag="rz", name=f"rz_{g}_{qh}_{h}")
                    nc.vector.reciprocal_approx_fast(out=rz, in_=zc)
                    ac = rzp.tile([D, 512], F32, tag="ac", name=f"ac_{g}_{qh}_{h}")
                    nc.vector.tensor_copy(out=ac, in_=a[0:D, hl])
                    rzb = rzp.tile([D, 512], F32, tag="rzb", name=f"rzb_{g}_{qh}_{h}")
                    nc.gpsimd.partition_broadcast(rzb, rz)
                    nc.vector.tensor_mul(
                        out=attnT[off : off + D, jt, 1024 * qh + 512 * h : 1024 * qh + 512 * (h + 1)],
                        in0=ac,
                        in1=rzb,
                    )

            npairs = NT // 2

            def do_pv_pair(idx_pair):
                # PV for key-tile pair p of block blk, fp8 DoubleRow
                blk, p = idx_pair
                g, qh = ORDER[blk]
                epp = eps.pop(idx_pair)
                for t in range(2):
                    nc.tensor.matmul(
                        accs[(g, qh)][0 : D + 1, 512 * t : 512 * (t + 1)],
                        lhsT=v_sb[:, 2 * p : 2 * p + 2, g, 0 : D + 1],
                        rhs=epp[:, 0:2, 512 * t : 512 * (t + 1)],
                        start=(p == 0),
                        stop=(p == npairs - 1),
                        perf_mode=DR,
                    )
                if p == npairs - 1:
                    normalize(g, qh)

            def do_pv_single(idx):
                pg, pqh, pi = units[idx]
                epp = eps.pop(idx)
                for t in range(2):
                    nc.tensor.matmul(
                        accs[(pg, pqh)][0 : D + 1, 512 * t : 512 * (t + 1)],
                        lhsT=v_sb[:, pi, pg, 0 : D + 1],
                        rhs=epp[:, 512 * t : 512 * (t + 1)],
                        start=(pi == 0),
                        stop=(pi == NT - 1),
                    )
                if pi == NT - 1:
                    normalize(pg, pqh)

            pv_q = []  # ready PV work items

            for idx, (g, qh, i) in enumerate(units):
                jt, off = g // 2, D * (g % 2)
                # drain one ready PV (runs 1-2 units behind the exp stream)
                if pv_q:
                    item = pv_q.pop(0)
                    if USE_DR_PV:
                        do_pv_pair(item)
                    else:
                        do_pv_single(item)
                if i == 0:
                    accs[(g, qh)] = acc_ps.tile(
                        [128, 1024], F32, tag="acc", name=f"acc_{g}_{qh}"
                    )
                kh = kT[jt][off : off + D, 128 * i : 128 * (i + 1)]
                sp = score_ps.tile([128, 1024], F32, tag="sp", name=f"sp_{idx}")
                for t in range(2):
                    nc.tensor.matmul(
                        sp[:, 512 * t : 512 * (t + 1)],
                        lhsT=kh,
                        rhs=qT[jt][
                            off : off + D,
                            1024 * qh + 512 * t : 1024 * qh + 512 * (t + 1),
                        ],
                        start=True,
                        stop=True,
                    )
                if USE_DR_PV:
                    blk, p = idx // NT, (idx % NT) // 2
                    if i % 2 == 0:
                        eps[(blk, p)] = expp.tile(
                            [128, 2, 1024], FP8, tag="ep", name=f"ep_{blk}_{p}"
                        )
                    nc.scalar.activation(eps[(blk, p)][:, i % 2, :], sp, Act.Exp)
                    if i % 2 == 1:
                        pv_q.append((blk, p))
                else:
                    ep = expp.tile([128, 1024], BF16, tag="ep", name=f"ep_{idx}")
                    nc.scalar.activation(ep, sp, Act.Exp)
                    eps[idx] = ep
                    pv_q.append(idx)
                for f in fills.get(idx, ()):
                    emit_fill(f)
            while pv_q:
                item = pv_q.pop(0)
                if USE_DR_PV:
                    do_pv_pair(item)
                else:
                    do_pv_single(item)

            # ---- tail: c=1 out-proj for tokens 0-1023 ----
            # junk burst runs on the PE during the final normalize (DVE/
            # GpSimd) and recovers the clock gate for the tail matmuls
            for _ in range(10):
                emit_fill(("junk",))
            for t in range(8):
                emit_c1(t, tail=True)

    nc.compile()
    return nc


_CACHE = {}


def _get_nc():
    if "nc" not in _CACHE:
        _CACHE["nc"] = build_nc()
    return _CACHE["nc"]


def make_in_maps(x, w_qkv, b_qkv, w_proj, b_proj):
    import ml_dtypes

    bf16 = ml_dtypes.bfloat16
    x = np.asarray(x, dtype=np.float32)
    w_qkv = np.asarray(w_qkv, dtype=np.float32)
    b_qkv = np.asarray(b_qkv, dtype=np.float32)
    w_proj = np.asarray(w_proj, dtype=np.float32)
    in_maps = []
    for c in range(8):
        b, hg = c // 2, c % 2
        s = 256 * hg
        w_slice = np.hstack(
            [
                w_qkv[:, s : s + 256],
                w_qkv[:, 512 + s : 512 + s + 256],
                w_qkv[:, 1024 + s : 1024 + s + 256],
            ]
        )
        b_slice = np.concatenate(
            [
                b_qkv[s : s + 256],
                b_qkv[512 + s : 512 + s + 256],
                b_qkv[1024 + s : 1024 + s + 256],
            ]
        )
        in_maps.append(
            {
                "x_in": np.ascontiguousarray(x[b].astype(bf16).T),
                "w_in": np.ascontiguousarray(w_slice.astype(bf16)),
                "b_in": np.ascontiguousarray(b_slice.astype(np.float32)),
                "wp_in": np.ascontiguousarray(w_proj[s : s + 256, :].astype(bf16)),
            }
        )
    return in_maps


def assemble(results, b_proj):
    full = np.empty((B, T, H), dtype=np.float32)
    for b in range(B):
        full[b] = (
            results[2 * b]["out"].astype(np.float32)
            + results[2 * b + 1]["out"].astype(np.float32)
            + np.asarray(b_proj, dtype=np.float32)[None, :]
        )
    return full


def kernel(x, w_qkv, b_qkv, w_proj, b_proj):
    from concourse.bass_utils import run_bass_kernel_spmd

    nc = _get_nc()
    in_maps = make_in_maps(x, w_qkv, b_qkv, w_proj, b_proj)
    res = run_bass_kernel_spmd(nc, in_maps, core_ids=list(range(8)))
    return assemble(res.results, b_proj)


# revision 15
# speedup vs baseline: 1.0264x; 1.0118x over previous
"""Trainium2 Bass kernel for an attention block (B=4, T=2048, H=512, 8 heads).

Sharding: 8 cores = 4 batches x 2 head-groups (4 heads each). Core c handles
batch c//2 and heads [4*(c%2), 4*(c%2)+4) over the FULL 2048-token context.
Each core emits a PARTIAL output projection (its 256 attn features x its
w_proj row-slice); the host sums the two partials per batch and adds b_proj.

v2 schedule, built around two measured facts:
  - ScalarE exp over [4 heads, 2048 q, 2048 k] is 128 ACTIVATEs of [128,1024]
    at ~1.09us each = ~140us. That stream is the floor; nothing else may ever
    block it.
  - The PE HAM clock gate halves the PE clock whenever the PE array has idle
    slack in its 3.4us activity window. At 1.2GHz the bf16 score+PV work
    (2048 cols/unit) exceeds the exp period (1147ns) and the exp stream
    starves. The baseline oscillated between these states and lost 60-100us.

  Fixes:
  - PV runs as fp8e4 DoubleRow over key-tile PAIRS (contraction 256/pass):
    512 PE cols/unit instead of 1024. exp output is written fp8 directly by
    the ACT (free cast); V is drained to fp8 by the DVE (free cast). Scores
    stay bf16 (K=64 contraction gains nothing from fp8).
  - Block order [g0qh0 g1qh0 g0qh1 g1qh1 g2qh1 g3qh1 g2qh0 g3qh0] over
    1024-wide query halves: heads 0,1 finish all tokens by unit 64, so the
    c=0 half of the output projection streams as PE filler from unit 66; the
    c=1 half for tokens 1024-2047 streams from unit 97 (with DMA-out); only
    tokens 0-1023's c=1 half remains for the tail.
  - All remaining QKV projections drip as deadline-scheduled fills, ~1 per
    unit, spread across the whole stream instead of front-loaded bursts.
    Optional junk matmuls pad the thin late blocks to keep the HAM warm.
  - PSUM: 2 score bufs (4 banks) + [65,1024] accumulator (2 banks, ones-row
    softmax denominator trick) + fill ping-pong (2 banks) = all 8.
"""

import sys

sys.path.insert(0, "/opt/trn_rl_repo")

from contextlib import ExitStack

import numpy as np

import concourse.bass as bass
import concourse.tile as tile
from concourse import bacc, mybir

F32 = mybir.dt.float32
BF16 = mybir.dt.bfloat16
FP8 = mybir.dt.float8e4
Act = mybir.ActivationFunctionType
DR = mybir.MatmulPerfMode.DoubleRow

B, T, H = 4, 2048, 512
HEADS = 8
D = H // HEADS  # 64
GC = 4  # heads per core
HC = H // 128  # 4 x-feature chunks
NT = T // 128  # 16 key tiles
NTT = T // 512  # 4 token 512-chunks
SCALE = float(D) ** -0.5
DP = 68  # padded v row stride (fp8 bytes): 64 d + ones + pad, %16 == 0

USE_DR_PV = True  # fp8e4 DoubleRow PV (pairs of key tiles)
JUNK = 0  # junk matmuls per unit in fill-starved units (off: measured net loss)

# block order: (g, qh) over 1024-token query halves
ORDER = [(0, 0), (1, 0), (0, 1), (1, 1), (2, 1), (3, 1), (2, 0), (3, 0)]


def build_nc():
    nc = bacc.Bacc("TRN2", target_bir_lowering=False, debug=False, num_devices=8)

    x_in = nc.dram_tensor("x_in", [H, T], BF16, kind="ExternalInput").ap()  # x.T
    w_in = nc.dram_tensor("w_in", [H, 3 * 256], BF16, kind="ExternalInput").ap()
    # bias layout: [bq*SCALE (256) | bk (256)] fp32, bv (256) fp32
    b_in = nc.dram_tensor("b_in", [3 * 256], F32, kind="ExternalInput").ap()
    wp_in = nc.dram_tensor("wp_in", [256, H], BF16, kind="ExternalInput").ap()
    out = nc.dram_tensor("out", [T, H], BF16, kind="ExternalOutput").ap()

    with tile.TileContext(nc) as tc, ExitStack() as ctx:
        per = ctx.enter_context(tc.tile_pool(name="persist", bufs=1))

        w_sb = per.tile([128, HC, 3 * 256], BF16)
        wp_sb = per.tile([128, 2, H], BF16)
        bqk_sb = per.tile([128, 4], F32)  # q bias chunks 0-1, k bias chunks 2-3
        bv_row = per.tile([1, 256], F32)
        bv_bc = per.tile([128, 256], F32)
        dummy = per.tile([1, 8], F32)
        warm = per.tile([128, 128], BF16)

        xT = per.tile([128, HC, T], BF16)
        kT = [per.tile([128, T], BF16, name=f"kT_{j}") for j in range(2)]
        qT = [per.tile([128, T], BF16, name=f"qT_{j}") for j in range(2)]
        v_dt = FP8 if USE_DR_PV else BF16
        v_sb = per.tile([128, NT, GC, DP], v_dt)
        attnT = per.tile([128, 2, T], BF16)
        part_sb = per.tile([128, NT, 512], F32)  # c=0 out-proj partials

        # ---- phase A ----
        with (
            tc.tile_pool(name="qkps", bufs=4, space="PSUM") as qkps,
            tc.tile_pool(name="warmps", bufs=1, space="PSUM") as warmps,
        ):
            # x is pre-transposed on the host. Few, big DMAs: w first on scalar
            # (the first matmuls need it), x quarters split across queues.
            def x_dma(eng, clo, tlo, thi):
                eng.dma_start(
                    out=xT[:, clo : clo + 2, tlo:thi],
                    in_=x_in[128 * clo : 128 * (clo + 2), tlo:thi].rearrange(
                        "(c p) t -> p c t", p=128
                    ),
                )

            nc.scalar.dma_start(
                out=w_sb[:, :, 0:512],
                in_=w_in[:, 0:512].rearrange("(c p) j -> p c j", p=128),
            )
            nc.scalar.dma_start(
                out=w_sb[:, :, 512:768],
                in_=w_in[:, 512:768].rearrange("(c p) j -> p c j", p=128),
            )
            nc.gpsimd.dma_start(
                out=bqk_sb, in_=b_in[0:512].rearrange("(c p) -> p c", p=128)
            )
            nc.gpsimd.dma_start(out=bv_row, in_=b_in[512:768].unsqueeze(0))
            x_dma(nc.sync, 0, 0, 512)
            x_dma(nc.gpsimd, 2, 0, 512)
            x_dma(nc.sync, 0, 512, 1024)
            x_dma(nc.gpsimd, 2, 512, 1024)
            x_dma(nc.sync, 2, 1024, 2048)
            x_dma(nc.scalar, 0, 1024, 2048)
            nc.scalar.dma_start(
                out=wp_sb, in_=wp_in.rearrange("(c p) j -> p c j", p=128)
            )
            # junk matmuls warm the PE clock gate (HAM) during the DMA wait so
            # the first real projections run at 2.4 GHz
            nc.vector.memset(warm, 0.0)
            wps = warmps.tile([128, 128], F32, tag="w", name="warmps")
            for _ in range(48):
                nc.tensor.matmul(wps, lhsT=warm, rhs=warm, start=True, stop=True)
            # preload the exp table while DMAs stream
            nc.gpsimd.memset(dummy, 0.0)
            nc.scalar.activation(dummy, dummy, Act.Exp)
            nc.gpsimd.partition_broadcast(bv_bc, bv_row)
            nc.vector.memset(v_sb[:, :, :, D : D + 1], 1.0)

            # K/Q projection group: 4 matmuls + a drain; emitted in halves
            # (c-chunk pairs) so fills spread smoothly.
            def kq_mm(kind, jt, tt, ps, cs):
                col0 = 256 + 128 * jt if kind == "k" else 128 * jt
                for c in cs:
                    nc.tensor.matmul(
                        ps[:, 0:512],
                        lhsT=w_sb[:, c, col0 : col0 + 128],
                        rhs=xT[:, c, 512 * tt : 512 * (tt + 1)],
                        start=(c == 0),
                        stop=(c == HC - 1),
                    )

            def kq_drain(kind, jt, tt, ps):
                if kind == "k":
                    nc.vector.tensor_scalar(
                        out=kT[jt][:, 512 * tt : 512 * (tt + 1)],
                        in0=ps[:, 0:512],
                        scalar1=bqk_sb[:, 2 + jt : 3 + jt],
                        scalar2=None,
                        op0=mybir.AluOpType.add,
                    )
                else:
                    nc.vector.tensor_scalar(
                        out=qT[jt][:, 512 * tt : 512 * (tt + 1)],
                        in0=ps[:, 0:512],
                        scalar1=bqk_sb[:, jt : jt + 1],
                        scalar2=SCALE,
                        op0=mybir.AluOpType.add,
                        op1=mybir.AluOpType.mult,
                    )

            # V for head range [2*ghalf, 2*ghalf+2) over `ks` 128-token tiles
            # of 512-chunk tt: 4 matmuls per tile, one strided drain.
            def v_group(tt, ghalf, ps, ks):
                for k in ks:
                    i = 4 * tt + k
                    for c in range(HC):
                        nc.tensor.matmul(
                            ps[:, 128 * (k % 4) : 128 * (k % 4) + 128],
                            lhsT=xT[:, c, 128 * i : 128 * (i + 1)],
                            rhs=w_sb[:, c, 512 + 128 * ghalf : 512 + 128 * (ghalf + 1)],
                            start=(c == 0),
                            stop=(c == HC - 1),
                        )
                nk = len(ks)
                k0 = ks[0]
                nc.vector.tensor_add(
                    out=v_sb[
                        :, 4 * tt + k0 : 4 * tt + k0 + nk, 2 * ghalf : 2 * ghalf + 2, 0:D
                    ],
                    in0=ps[:, 128 * (k0 % 4) : 128 * ((k0 % 4) + nk)].rearrange(
                        "p (k g d) -> p k g d", k=nk, g=2
                    ),
                    in1=bv_bc[:, 128 * ghalf : 128 * (ghalf + 1)]
                    .rearrange("p (g d) -> p g d", g=2)
                    .unsqueeze(1)
                    .broadcast_to([128, nk, 2, D]),
                )

            # phase A pre-work: everything unit 0/1 needs
            psA = qkps.tile([128, 512], F32, tag="g", name="ps_q00")
            kq_mm("q", 0, 0, psA, range(HC))
            kq_drain("q", 0, 0, psA)
            psB = qkps.tile([128, 512], F32, tag="g", name="ps_q01")
            kq_mm("q", 0, 1, psB, range(HC))
            kq_drain("q", 0, 1, psB)
            psC = qkps.tile([128, 512], F32, tag="g", name="ps_k00")
            kq_mm("k", 0, 0, psC, range(HC))
            kq_drain("k", 0, 0, psC)
            psD = qkps.tile([128, 512], F32, tag="g", name="ps_v00")
            v_group(0, 0, psD, (0, 1, 2, 3))

        # ---- phase B: attention ----
        units = [(g, qh, i) for (g, qh) in ORDER for i in range(NT)]

        # fill schedule: unit -> list of emission descriptors
        #   ("kqA", kind, jt, tt) / ("kqB", kind, jt, tt): group halves
        #   ("v2", tt, ghalf, k0): 2-tile v half-group
        #   ("c0", t) / ("c1", t): out-proj halves for token tile t
        #   ("junk",): one [128,512] junk matmul (HAM warmth)
        fills = {u: [] for u in range(len(units))}

        def place(u, *item):
            fills[u].append(item)

        def place_group(u, kind, jt, tt):
            place(u, "kqA", kind, jt, tt)
            place(u + 1, "kqB", kind, jt, tt)

        # HAM recovery-burst schedule. Measured behavior: once the clock gate
        # drops to 4/8 it only recovers after a ~3.4us window of STALL-FREE
        # PE saturation; the steady score/PV stream micro-stalls on the exp
        # pipeline and never recovers. So every 8 units carries one ~4096-
        # cycle back-to-back burst (a double projection group where real work
        # remains, junk matmuls after) sized to be absorbed by the 2-deep
        # score pipeline when the clock is already at 8/8.
        # B0: forced JIT fills (kT0/v-gh0 stream just ahead of the scores).
        place_group(0, "k", 0, 1)
        place(2, "v2", 1, 0, 0)
        place(3, "v2", 1, 0, 2)
        place_group(4, "k", 0, 2)
        place(6, "v2", 2, 0, 0)
        place(7, "v2", 2, 0, 2)
        place_group(8, "k", 0, 3)
        place(10, "v2", 3, 0, 0)
        place(11, "v2", 3, 0, 2)

        def burst2(u, spec1, spec2):
            # spec: ("q"/"k", jt, tt) or ("v", tt, ghalf)
            for kind, a1, a2 in (spec1, spec2):
                if kind == "v":
                    place(u, "v2", a1, a2, 0)
                    place(u, "v2", a1, a2, 2)
                else:
                    place(u, "kqA", kind, a1, a2)
                    place(u, "kqB", kind, a1, a2)

        burst2(16, ("q", 0, 2), ("q", 0, 3))
        burst2(24, ("q", 1, 2), ("q", 1, 3))
        burst2(32, ("k", 1, 0), ("v", 0, 1))
        burst2(40, ("k", 1, 1), ("v", 1, 1))
        burst2(48, ("k", 1, 2), ("v", 2, 1))
        burst2(56, ("k", 1, 3), ("v", 3, 1))
        burst2(64, ("q", 1, 0), ("q", 1, 1))
        burst_units = (70, 76, 82, 88, 94, 100, 106, 112, 118, 123)
        for u in burst_units:
            for _ in range(10):
                place(u, "junk")

        def place_avoiding(u0, *item):
            u = u0
            while u in burst_units:
                u += 1
            place(u, *item)

        # c0 out-proj partials (need all of heads 0,1: ready ~u66)
        for t in range(NT):
            place_avoiding(66 + t, "c0", t)
        # c1 finals for tokens 1024-2047 (tiles 8-15, ready ~u97) + DMA out
        for t in range(8, NT):
            place_avoiding(97 + (t - 8), "c1", t)

        with (
            tc.tile_pool(name="expp", bufs=15 if USE_DR_PV else 30) as expp,
            tc.tile_pool(name="rz", bufs=2) as rzp,
            tc.tile_pool(name="ostage", bufs=2) as ostage,
            tc.tile_pool(name="scoreps", bufs=2, space="PSUM") as score_ps,
            tc.tile_pool(name="accps", bufs=1, space="PSUM") as acc_ps,
            tc.tile_pool(name="fillps", bufs=2, space="PSUM") as fill_ps,
        ):
            pending_kq = {}
            ot_state = {}

            def emit_c0(t):
                ps = fill_ps.tile([128, 512], F32, tag="f", name=f"c0_{t}")
                nc.tensor.matmul(
                    ps,
                    lhsT=attnT[:, 0, 128 * t : 128 * (t + 1)],
                    rhs=wp_sb[:, 0, :],
                    start=True,
                    stop=True,
                )
                nc.vector.tensor_copy(out=part_sb[:, t, :], in_=ps)

            tail_sp = {}

            def emit_c1(t):
                ps = fill_ps.tile([128, 512], F32, tag="f", name=f"c1_{t}")
                nc.tensor.matmul(
                    ps,
                    lhsT=attnT[:, 1, 128 * t : 128 * (t + 1)],
                    rhs=wp_sb[:, 1, :],
                    start=True,
                    stop=True,
                )
                tp, k = t // 2, t % 2
                if k == 0:
                    ot_state[tp] = ostage.tile(
                        [128, 2, H], BF16, tag="ot", name=f"ot_{tp}"
                    )
                nc.vector.tensor_add(
                    out=ot_state[tp][:, k, :], in0=ps, in1=part_sb[:, t, :]
                )
                if k == 1:
                    nc.sync.dma_start(
                        out=out[256 * tp : 256 * (tp + 1), :].rearrange(
                            "(i p) j -> p i j", p=128
                        ),
                        in_=ot_state[tp],
                    )

            def emit_fill(f):
                kind = f[0]
                if kind == "kqA":
                    _, kq, jt, tt = f
                    ps = fill_ps.tile([128, 512], F32, tag="f", name=f"f{kq}{jt}{tt}")
                    pending_kq[(kq, jt, tt)] = ps
                    kq_mm(kq, jt, tt, ps, (0, 1))
                elif kind == "kqB":
                    _, kq, jt, tt = f
                    ps = pending_kq.pop((kq, jt, tt))
                    kq_mm(kq, jt, tt, ps, (2, 3))
                    kq_drain(kq, jt, tt, ps)
                elif kind == "v2":
                    _, tt, ghalf, k0 = f
                    ps = fill_ps.tile([128, 512], F32, tag="f", name=f"fv{tt}{ghalf}{k0}")
                    v_group(tt, ghalf, ps, (k0, k0 + 1))
                elif kind == "c0":
                    emit_c0(f[1])
                elif kind == "c1":
                    emit_c1(f[1])
                else:  # junk
                    ps = fill_ps.tile([128, 512], F32, tag="f", name="junk")
                    nc.tensor.matmul(
                        ps,
                        lhsT=w_sb[:, 0, 0:128],
                        rhs=xT[:, 0, 0:512],
                        start=True,
                        stop=True,
                    )

            accs = {}
            eps = {}

            def normalize(g, qh, use_scalar=False):
                # acc rows 0-63 = attn values, row 64 = Z (ones-row trick).
                # Split in 512-halves so zc/recip/broadcast/mul pipeline. The
                # Z row is copied to partition 0 first (composite DVE ops
                # mishandle nonzero input base partitions). For the final
                # block the ac copies go on the now-idle ScalarE so the DVE
                # only carries the recip/mul chain.
                jt, off = g // 2, D * (g % 2)
                a = accs[(g, qh)]
                for h in range(2):
                    hl = slice(512 * h, 512 * (h + 1))
                    zc = rzp.tile([1, 512], F32, tag="zc", name=f"zc_{g}_{qh}_{h}")
                    nc.vector.tensor_copy(out=zc, in_=a[D : D + 1, hl])
                    rz = rzp.tile([1, 512], F32, tag="rz", name=f"rz_{g}_{qh}_{h}")
                    nc.vector.reciprocal_approx_fast(out=rz, in_=zc)
                    ac = rzp.tile([D, 512], F32, tag="ac", name=f"ac_{g}_{qh}_{h}")
                    if use_scalar:
                        nc.scalar.copy(out=ac, in_=a[0:D, hl])
                    else:
                        nc.vector.tensor_copy(out=ac, in_=a[0:D, hl])
                    rzb = rzp.tile([D, 512], F32, tag="rzb", name=f"rzb_{g}_{qh}_{h}")
                    nc.gpsimd.partition_broadcast(rzb, rz)
                    nc.vector.tensor_mul(
                        out=attnT[off : off + D, jt, 1024 * qh + 512 * h : 1024 * qh + 512 * (h + 1)],
                        in0=ac,
                        in1=rzb,
                    )

            npairs = NT // 2

            def do_pv_pair(idx_pair):
                # PV for key-tile pair p of block blk, fp8 DoubleRow
                blk, p = idx_pair
                g, qh = ORDER[blk]
                epp = eps.pop(idx_pair)
                for t in range(2):
                    nc.tensor.matmul(
                        accs[(g, qh)][0 : D + 1, 512 * t : 512 * (t + 1)],
                        lhsT=v_sb[:, 2 * p : 2 * p + 2, g, 0 : D + 1],
                        rhs=epp[:, 0:2, 512 * t : 512 * (t + 1)],
                        start=(p == 0),
                        stop=(p == npairs - 1),
                        perf_mode=DR,
                    )
                if p == npairs - 1:
                    normalize(g, qh, use_scalar=(blk == len(ORDER) - 1))

            def do_pv_single(idx):
                pg, pqh, pi = units[idx]
                epp = eps.pop(idx)
                for t in range(2):
                    nc.tensor.matmul(
                        accs[(pg, pqh)][0 : D + 1, 512 * t : 512 * (t + 1)],
                        lhsT=v_sb[:, pi, pg, 0 : D + 1],
                        rhs=epp[:, 512 * t : 512 * (t + 1)],
                        start=(pi == 0),
                        stop=(pi == NT - 1),
                    )
                if pi == NT - 1:
                    normalize(pg, pqh, use_scalar=(idx >= len(units) - 1))

            pv_q = []  # ready PV work items

            for idx, (g, qh, i) in enumerate(units):
                jt, off = g // 2, D * (g % 2)
                # drain one ready PV (runs 1-2 units behind the exp stream)
                if pv_q:
                    item = pv_q.pop(0)
                    if USE_DR_PV:
                        do_pv_pair(item)
                    else:
                        do_pv_single(item)
                if i == 0:
                    accs[(g, qh)] = acc_ps.tile(
                        [128, 1024], F32, tag="acc", name=f"acc_{g}_{qh}"
                    )
                kh = kT[jt][off : off + D, 128 * i : 128 * (i + 1)]
                sp = score_ps.tile([128, 1024], F32, tag="sp", name=f"sp_{idx}")
                for t in range(2):
                    nc.tensor.matmul(
                        sp[:, 512 * t : 512 * (t + 1)],
                        lhsT=kh,
                        rhs=qT[jt][
                            off : off + D,
                            1024 * qh + 512 * t : 1024 * qh + 512 * (t + 1),
                        ],
                        start=True,
                        stop=True,
                    )
                if USE_DR_PV:
                    blk, p = idx // NT, (idx % NT) // 2
                    if i % 2 == 0:
                        eps[(blk, p)] = expp.tile(
                            [128, 2, 1024], FP8, tag="ep", name=f"ep_{blk}_{p}"
                        )
                    nc.scalar.activation(eps[(blk, p)][:, i % 2, :], sp, Act.Exp)
                    if i % 2 == 1:
                        pv_q.append((blk, p))
                else:
                    ep = expp.tile([128, 1024], BF16, tag="ep", name=f"ep_{idx}")
                    nc.scalar.activation(ep, sp, Act.Exp)
                    eps[idx] = ep
                    pv_q.append(idx)
                for f in fills.get(idx, ()):
                    emit_fill(f)
            while pv_q:
                item = pv_q.pop(0)
                if USE_DR_PV:
                    do_pv_pair(item)
                else:
                    do_pv_single(item)

            # ---- tail: out-proj for tokens 0-1023 ----
            # Both halves accumulate in PSUM and drain via the now-idle
            # ScalarE, so the tail needs no DVE adds at all. A junk burst
            # runs on the PE during the final normalize to recover the
            # clock gate for these matmuls.
            for _ in range(10):
                emit_fill(("junk",))
            for t in range(8):
                if t % 2 == 0:
                    tail_sp[t // 2] = score_ps.tile(
                        [128, 1024], F32, tag="sp", name=f"c1t_{t // 2}"
                    )
                    ot_state[100 + t // 2] = ostage.tile(
                        [128, 2, H], BF16, tag="ot", name=f"oT_{t // 2}"
                    )
                ps = tail_sp[t // 2][:, 512 * (t % 2) : 512 * (t % 2) + 512]
                for c in range(2):
                    nc.tensor.matmul(
                        ps,
                        lhsT=attnT[:, c, 128 * t : 128 * (t + 1)],
                        rhs=wp_sb[:, c, :],
                        start=(c == 0),
                        stop=(c == 1),
                    )
                nc.scalar.copy(out=ot_state[100 + t // 2][:, t % 2, :], in_=ps)
                if t % 2 == 1:
                    eng = nc.sync if (t // 2) % 2 == 0 else nc.scalar
                    eng.dma_start(
                        out=out[256 * (t // 2) : 256 * (t // 2) + 256, :].rearrange(
                            "(i p) j -> p i j", p=128
                        ),
                        in_=ot_state[100 + t // 2],
                    )

    nc.compile()
    return nc


_CACHE = {}


def _get_nc():
    if "nc" not in _CACHE:
        _CACHE["nc"] = build_nc()
    return _CACHE["nc"]


def make_in_maps(x, w_qkv, b_qkv, w_proj, b_proj):
    import ml_dtypes

    bf16 = ml_dtypes.bfloat16
    x = np.asarray(x, dtype=np.float32)
    w_qkv = np.asarray(w_qkv, dtype=np.float32)
    b_qkv = np.asarray(b_qkv, dtype=np.float32)
    w_proj = np.asarray(w_proj, dtype=np.float32)
    in_maps = []
    for c in range(8):
        b, hg = c // 2, c % 2
        s = 256 * hg
        w_slice = np.hstack(
            [
                w_qkv[:, s : s + 256],
                w_qkv[:, 512 + s : 512 + s + 256],
                w_qkv[:, 1024 + s : 1024 + s + 256],
            ]
        )
        b_slice = np.concatenate(
            [
                b_qkv[s : s + 256],
                b_qkv[512 + s : 512 + s + 256],
                b_qkv[1024 + s : 1024 + s + 256],
            ]
        )
        in_maps.append(
            {
                "x_in": np.ascontiguousarray(x[b].astype(bf16).T),
                "w_in": np.ascontiguousarray(w_slice.astype(bf16)),
                "b_in": np.ascontiguousarray(b_slice.astype(np.float32)),
                "wp_in": np.ascontiguousarray(w_proj[s : s + 256, :].astype(bf16)),
            }
        )
    return in_maps


def assemble(results, b_proj):
    full = np.empty((B, T, H), dtype=np.float32)
    for b in range(B):
        full[b] = (
            results[2 * b]["out"].astype(np.float32)
            + results[2 * b + 1]["out"].astype(np.float32)
            + np.asarray(b_proj, dtype=np.float32)[None, :]
        )
    return full


def kernel(x, w_qkv, b_qkv, w_proj, b_proj):
    from concourse.bass_utils import run_bass_kernel_spmd

    nc = _get_nc()
    in_maps = make_in_maps(x, w_qkv, b_qkv, w_proj, b_proj)
    res = run_bass_kernel_spmd(nc, in_maps, core_ids=list(range(8)))
    return assemble(res.results, b_proj)
